# revision 24
# baseline (speedup 1.0000x reference)
"""Local-strided block-sparse paged attention (decode) on 8 Trainium2 cores.

Strategy (memory-bound -> minimize device HBM bytes and DMA/descriptor serialization):
- Host resolves the per-(batch, q-head) CSR rows, then DEDUPLICATES the kv
  blocks across the 4 q-heads of each kv-head group: one gathered K/V panel
  per (b, kv_head) union, with per-head multiplicative masks restoring each
  head's exact row + causal masking.
- Panels are fp16 (halves bytes; ~5e-4 rel err, tolerance 2e-2).
- The 64 panels are assigned to 8 cores x 8 slots sorted by size, so the
  SPMD slot padding (same program on all cores) wastes little bandwidth.
- Two DMAs per slot: [K|mask] issued by the Sync engine, [V] by the Scalar
  engine -- descriptor generation (~630ns/DMA) runs on both engines in
  parallel instead of serializing on Sync.
- Device per slot (all 4 heads batched per matmul):
    QK:   per 128-token chunk: stationary K-chunk [128d,128t] (FWL),
          moving q [128,4] -> scores[t, 4h] in PSUM
    p   = exp(scores*scale) (ACT, fp16 out) * mask (DVE fp16)
    PV:   per chunk: stationary V-chunk [128t,128d] (FWL), moving p-chunk
          [128,4] -> accumulates out [128d, 4h] in PSUM
    den:  ones[128,1]^T @ p -> per-(chunk,head) sums; host reduces + divides
- Outputs batched into 2 final DMAs; host does the final normalization.
"""
import numpy as np

B, H, KVH, D, X = 16, 16, 4, 128, 4
GRP = H // KVH              # q heads per kv head
BLK, MAXB = 16, 256
NC_CORES = 8
NSLOTS = (B * KVH) // NC_CORES   # panels per core
SM_SCALE = 1.0 / float(np.sqrt(D))

_PROG_CACHE = {}


def _build_device_program(slot_nch):
    import concourse.bacc as bacc
    import concourse.mybir as mybir
    from concourse.tile import TileContext

    f32 = mybir.dt.float32
    f16 = mybir.dt.float16
    nc = bacc.Bacc("TRN2", target_bir_lowering=False)
    # per-slot [K | mask] and [V] panels
    km = [nc.dram_tensor(f"km{s}", [128, slot_nch[s] * 132], f16, kind="ExternalInput")
          for s in range(NSLOTS)]
    vv = [nc.dram_tensor(f"vv{s}", [128, slot_nch[s] * 128], f16, kind="ExternalInput")
          for s in range(NSLOTS)]
    qd = nc.dram_tensor("qd", [128, 4 * NSLOTS], f16, kind="ExternalInput")
    oud = nc.dram_tensor("oud", [128, 4 * NSLOTS], f32, kind="ExternalOutput")
    SUMW = sum(nch * 4 for nch in slot_nch)
    sud = nc.dram_tensor("sud", [1, SUMW], f32, kind="ExternalOutput")

    with TileContext(nc) as tc:
        with (
            tc.tile_pool(name="data", bufs=1) as dp,
            tc.tile_pool(name="work", bufs=3) as wp,
            tc.tile_pool(name="ps_sc", bufs=2, space="PSUM") as psc,
            tc.tile_pool(name="ps_ov", bufs=2, space="PSUM") as pov,
            tc.tile_pool(name="ps_ds", bufs=2, space="PSUM") as pds,
        ):
            qt = dp.tile([128, 4 * NSLOTS], f16, tag="q")
            nc.sync.dma_start(out=qt[:], in_=qd[:])
            ones = dp.tile([128, 1], f16, tag="ones")
            nc.vector.memset(ones[:], 1.0)
            osb = dp.tile([128, 4 * NSLOTS], f32, tag="osb")
            ssb = dp.tile([1, SUMW], f32, tag="ssb")

            kms, vvs = [], []
            for s in range(NSLOTS):
                kmt = dp.tile([128, slot_nch[s] * 132], f16, tag=f"km{s}")
                nc.sync.dma_start(out=kmt[:], in_=km[s][:])
                vvt = dp.tile([128, slot_nch[s] * 128], f16, tag=f"vv{s}")
                nc.scalar.dma_start(out=vvt[:], in_=vv[s][:])
                kms.append(kmt); vvs.append(vvt)

            ps = [None] * NSLOTS
            sum_off = [0] * NSLOTS
            off = 0
            for s in range(NSLOTS):
                sum_off[s] = off
                off += slot_nch[s] * 4

            def qk_softmax(s):
                nch = slot_nch[s]
                sc = psc.tile([128, nch * 4], f32, tag="sc")
                for c in range(nch):
                    nc.tensor.matmul(
                        sc[:, 4 * c:4 * c + 4],
                        kms[s][:, 128 * c:128 * (c + 1)],
                        qt[:, 4 * s:4 * s + 4],
                        start=True, stop=True,
                    )
                p0 = wp.tile([128, nch * 4], f16, tag="p0")
                nc.scalar.activation(
                    p0[:], sc[:], mybir.ActivationFunctionType.Exp,
                    scale=SM_SCALE,
                )
                p = wp.tile([128, nch * 4], f16, tag="p")
                nc.vector.tensor_mul(
                    p[:], p0[:], kms[s][:, nch * 128:nch * 132])
                ps[s] = p

            def pv(s):
                nch = slot_nch[s]
                ov = pov.tile([128, 4], f32, tag="ov")
                for c in range(nch):
                    nc.tensor.matmul(
                        ov[:],
                        vvs[s][:, 128 * c:128 * (c + 1)],
                        ps[s][:, 4 * c:4 * c + 4],
                        start=(c == 0), stop=(c == nch - 1),
                    )
                ds = pds.tile([1, nch * 4], f32, tag="ds")
                nc.tensor.matmul(ds[:], ones[:], ps[s][:], start=True, stop=True)
                nc.vector.tensor_copy(osb[:, 4 * s:4 * s + 4], ov[:])
                nc.vector.tensor_copy(
                    ssb[:, sum_off[s]:sum_off[s] + nch * 4], ds[:])
                ps[s] = None

            # software pipeline: PE does QK(s+1) while ACT/DVE produce p(s);
            # PE never waits on the softmax chain.
            qk_softmax(0)
            for s in range(1, NSLOTS):
                qk_softmax(s)
                pv(s - 1)
            pv(NSLOTS - 1)

            nc.sync.dma_start(out=oud[:], in_=osb[:])
            nc.scalar.dma_start(out=sud[:], in_=ssb[:])
    nc.compile()
    return nc


def _prep(q, k_cache, v_cache, block_tables, context_lens, layout_crow, layout_col):
    """Resolve CSR rows, dedup kv blocks per (b, kv-head), build panels."""
    q_pid = context_lens.astype(np.int64) - 1            # [B]
    pbid = q_pid // BLK
    h_idx = np.arange(H)
    start = layout_crow[h_idx[None, :], pbid[:, None]]   # [B,H]
    end = layout_crow[h_idx[None, :], pbid[:, None] + 1]

    panels = []  # (nch, b, kv, U, cols_per_head)
    for b in range(B):
        for kv in range(KVH):
            cols_h = []
            for dh in range(GRP):
                h = kv * GRP + dh
                cols_h.append(layout_col[h, start[b, h]:end[b, h]])
            U = np.unique(np.concatenate(cols_h))
            nch = max(1, -(-(len(U) * BLK) // 128))
            panels.append((nch, b, kv, U, cols_h))

    order = sorted(range(len(panels)), key=lambda i: -panels[i][0])
    slot_nch = [0] * NSLOTS
    assign = [[None] * NSLOTS for _ in range(NC_CORES)]
    for rank, pi in enumerate(order):
        core, s = rank % NC_CORES, rank // NC_CORES
        assign[core][s] = pi
        if core == 0:
            slot_nch[s] = panels[pi][0]
    slot_nch = tuple(slot_nch)

    in_maps = []
    meta = []    # per core: list of (b, kv) per slot
    tok16 = np.arange(BLK)
    for core in range(NC_CORES):
        im = {}
        mt_core = []
        qd = np.zeros((128, 4 * NSLOTS), np.float16)
        for s in range(NSLOTS):
            nch, b, kv, U, cols_h = panels[assign[core][s]]
            NT = slot_nch[s] * 128
            NU = len(U)
            phys = block_tables[b, U]

            kmt = np.zeros((128, slot_nch[s] * 132), np.float16)
            kb = k_cache[phys, kv]                       # [NU, 32, 16, 4]
            kmt[:, :NU * BLK] = kb.transpose(1, 3, 0, 2).reshape(128, NU * BLK)

            vb = v_cache[phys, kv]                       # [NU, 128, 16]
            v_t = np.zeros((NT, 128), np.float16)
            v_t[:NU * BLK] = vb.transpose(0, 2, 1).reshape(NU * BLK, 128)
            vvt = np.ascontiguousarray(
                v_t.reshape(slot_nch[s], 128, 128).transpose(1, 0, 2)
                .reshape(128, NT))

            mm = np.zeros((4, NT), np.float16)
            upos = U * BLK
            causal = (upos[:, None] + tok16[None, :]) <= q_pid[b]   # [NU,16]
            for dh in range(GRP):
                allowed = np.isin(U, cols_h[dh])[:, None] & causal
                mm[dh, :NU * BLK] = allowed.reshape(-1).astype(np.float16)
            kmt[:, NT:] = (
                mm.reshape(4, slot_nch[s], 128).transpose(2, 1, 0)
                .reshape(128, slot_nch[s] * 4))

            im[f"km{s}"] = kmt
            im[f"vv{s}"] = vvt
            qd[:, 4 * s:4 * s + 4] = q[b, kv * GRP:(kv + 1) * GRP].T
            mt_core.append((b, kv))
        im["qd"] = qd
        in_maps.append(im)
        meta.append(mt_core)
    return slot_nch, in_maps, meta


def kernel(q, k_cache, v_cache, block_tables, context_lens, layout_crow, layout_col):
    import os
    from concourse.bass_utils import run_bass_kernel_spmd

    q = np.asarray(q, np.float32)
    k_cache = np.asarray(k_cache, np.float32)
    v_cache = np.asarray(v_cache, np.float32)
    block_tables = np.asarray(block_tables, np.int32)
    context_lens = np.asarray(context_lens, np.int32)
    layout_crow = np.asarray(layout_crow, np.int32)
    layout_col = np.asarray(layout_col, np.int32)

    slot_nch, in_maps, meta = _prep(
        q, k_cache, v_cache, block_tables, context_lens, layout_crow, layout_col)

    nc = _PROG_CACHE.get(slot_nch)
    if nc is None:
        nc = _build_device_program(slot_nch)
        _PROG_CACHE[slot_nch] = nc

    res = run_bass_kernel_spmd(
        nc, in_maps, core_ids=list(range(NC_CORES)),
        trace=bool(os.environ.get("KERNEL_TRACE")),
    )
    global _LAST_RESULT
    _LAST_RESULT = res

    out = np.empty((B, H, D), np.float32)
    for core in range(NC_CORES):
        oud = res.results[core]["oud"]                   # [128, 4*NSLOTS]
        sud = res.results[core]["sud"][0]                # [SUMW]
        off = 0
        for s in range(NSLOTS):
            nch = slot_nch[s]
            b, kv = meta[core][s]
            den = sud[off:off + nch * 4].reshape(nch, 4).sum(0)   # [4]
            out[b, kv * GRP:(kv + 1) * GRP] = (oud[:, 4 * s:4 * s + 4] / den).T
            off += nch * 4
    return out


_LAST_RESULT = None


# revision 31
# speedup vs baseline: 1.0538x; 1.0538x over previous
"""Local-strided block-sparse paged attention (decode) on 8 Trainium2 cores.

Strategy (memory-bound -> minimize device HBM bytes and DMA/descriptor serialization):
- Host resolves the per-(batch, q-head) CSR rows, then DEDUPLICATES the kv
  blocks across the 4 q-heads of each kv-head group: one gathered K/V panel
  per (b, kv_head) union, with per-head multiplicative masks restoring each
  head's exact row + causal masking.
- Panels are fp16 (halves bytes; ~5e-4 rel err, tolerance 2e-2).
- The 64 panels are assigned to 8 cores x 8 slots sorted by size, so the
  SPMD slot padding (same program on all cores) wastes little bandwidth.
- Two DMAs per slot: [K|mask] issued by the Sync engine, [V] by the Scalar
  engine -- descriptor generation (~630ns/DMA) runs on both engines in
  parallel instead of serializing on Sync.
- Device per slot (all 4 heads batched per matmul):
    QK:   per 128-token chunk: stationary K-chunk [128d,128t] (FWL),
          moving q [128,4] -> scores[t, 4h] in PSUM
    p   = exp(scores*scale) (ACT, fp16 out) * mask (DVE fp16)
    PV:   per chunk: stationary V-chunk [128t,128d] (FWL), moving p-chunk
          [128,4] -> accumulates out [128d, 4h] in PSUM
    den:  ones[128,1]^T @ p -> per-(chunk,head) sums; host reduces + divides
- Outputs batched into 2 final DMAs; host does the final normalization.
"""
import numpy as np

B, H, KVH, D, X = 16, 16, 4, 128, 4
GRP = H // KVH              # q heads per kv head
BLK, MAXB = 16, 256
NC_CORES = 8
NSLOTS = (B * KVH) // NC_CORES   # panels per core
SM_SCALE = 1.0 / float(np.sqrt(D))

_PROG_CACHE = {}


def _build_device_program(slot_nch):
    import concourse.bacc as bacc
    import concourse.mybir as mybir
    from concourse.tile import TileContext

    f32 = mybir.dt.float32
    f16 = mybir.dt.float16
    nc = bacc.Bacc("TRN2", target_bir_lowering=False)
    # per-slot [K | mask] and [V] panels
    km = [nc.dram_tensor(f"km{s}", [128, slot_nch[s] * 132], f16, kind="ExternalInput")
          for s in range(NSLOTS)]
    vv = [nc.dram_tensor(f"vv{s}", [128, slot_nch[s] * 128], f16, kind="ExternalInput")
          for s in range(NSLOTS)]
    qd = nc.dram_tensor("qd", [128, 4 * NSLOTS + 1], f16, kind="ExternalInput")
    oud = nc.dram_tensor("oud", [128, 4 * NSLOTS], f32, kind="ExternalOutput")
    SUMW = sum(nch * 4 for nch in slot_nch)
    sud = nc.dram_tensor("sud", [1, SUMW], f32, kind="ExternalOutput")

    with TileContext(nc) as tc:
        with (
            tc.tile_pool(name="data", bufs=1) as dp,
            tc.tile_pool(name="work", bufs=3) as wp,
            tc.tile_pool(name="ps_sc", bufs=2, space="PSUM") as psc,
            tc.tile_pool(name="ps_ov", bufs=2, space="PSUM") as pov,
            tc.tile_pool(name="ps_ds", bufs=2, space="PSUM") as pds,
        ):
            qt = dp.tile([128, 4 * NSLOTS + 1], f16, tag="q")
            nc.sync.dma_start(out=qt[:], in_=qd[:])
            ones = qt[:, 4 * NSLOTS:4 * NSLOTS + 1]
            osb = dp.tile([128, 4 * NSLOTS], f32, tag="osb")
            ssb = dp.tile([1, SUMW], f32, tag="ssb")

            kms, vvs = [], []
            for s in range(NSLOTS):
                kmt = dp.tile([128, slot_nch[s] * 132], f16, tag=f"km{s}")
                nc.sync.dma_start(out=kmt[:], in_=km[s][:])
                vvt = dp.tile([128, slot_nch[s] * 128], f16, tag=f"vv{s}")
                nc.scalar.dma_start(out=vvt[:], in_=vv[s][:])
                kms.append(kmt); vvs.append(vvt)

            ps = [None] * NSLOTS
            sum_off = [0] * NSLOTS
            off = 0
            for s in range(NSLOTS):
                sum_off[s] = off
                off += slot_nch[s] * 4

            def qk_softmax(s):
                nch = slot_nch[s]
                sc = psc.tile([128, nch * 4], f32, tag="sc")
                for c in range(nch):
                    nc.tensor.matmul(
                        sc[:, 4 * c:4 * c + 4],
                        kms[s][:, 128 * c:128 * (c + 1)],
                        qt[:, 4 * s:4 * s + 4],
                        start=True, stop=True,
                    )
                p0 = wp.tile([128, nch * 4], f16, tag="p0")
                nc.scalar.activation(
                    p0[:], sc[:], mybir.ActivationFunctionType.Exp,
                    scale=SM_SCALE,
                )
                p = wp.tile([128, nch * 4], f16, tag="p")
                nc.vector.tensor_mul(
                    p[:], p0[:], kms[s][:, nch * 128:nch * 132])
                ps[s] = p

            def pv(s):
                nch = slot_nch[s]
                ov = pov.tile([128, 4], f32, tag="ov")
                for c in range(nch):
                    nc.tensor.matmul(
                        ov[:],
                        vvs[s][:, 128 * c:128 * (c + 1)],
                        ps[s][:, 4 * c:4 * c + 4],
                        start=(c == 0), stop=(c == nch - 1),
                    )
                ds = pds.tile([1, nch * 4], f32, tag="ds")
                nc.tensor.matmul(ds[:], ones, ps[s][:], start=True, stop=True)
                nc.vector.tensor_copy(osb[:, 4 * s:4 * s + 4], ov[:])
                nc.vector.tensor_copy(
                    ssb[:, sum_off[s]:sum_off[s] + nch * 4], ds[:])
                ps[s] = None

            # software pipeline: PE does QK(s+1) while ACT/DVE produce p(s);
            # PE never waits on the softmax chain.
            qk_softmax(0)
            for s in range(1, NSLOTS):
                qk_softmax(s)
                pv(s - 1)
            pv(NSLOTS - 1)

            nc.sync.dma_start(out=oud[:], in_=osb[:])
            nc.scalar.dma_start(out=sud[:], in_=ssb[:])
    nc.compile()
    return nc


def _build_device_program_raw(slot_nch):
    """Raw bacc (no TileContext): manual semaphores. Saves Tile's exit
    drain/barrier/sem-clear sequence (~4.8us) and part of its preamble."""
    import concourse.bacc as bacc
    import concourse.bass as bass
    import concourse.mybir as mybir
    from contextlib import ExitStack

    f32 = mybir.dt.float32
    f16 = mybir.dt.float16
    NS = NSLOTS
    nc = bacc.Bacc("TRN2", target_bir_lowering=False)
    km_d = [nc.dram_tensor(f"km{s}", [128, slot_nch[s] * 132], f16, kind="ExternalInput")
            for s in range(NS)]
    vv_d = [nc.dram_tensor(f"vv{s}", [128, slot_nch[s] * 128], f16, kind="ExternalInput")
            for s in range(NS)]
    qd_d = nc.dram_tensor("qd", [128, 4 * NS + 1], f16, kind="ExternalInput")
    oud_d = nc.dram_tensor("oud", [128, 4 * NS], f32, kind="ExternalOutput")
    SUMW = sum(nch * 4 for nch in slot_nch)
    sud_d = nc.dram_tensor("sud", [1, SUMW], f32, kind="ExternalOutput")
    sum_off = [0] * NS
    off = 0
    for s in range(NS):
        sum_off[s] = off
        off += slot_nch[s] * 4
    MAXW = max(slot_nch) * 4

    es = ExitStack()
    with es:
        qt = es.enter_context(nc.sbuf_tensor("qt", [128, 4 * NS + 1], f16))
        kmt = [es.enter_context(nc.sbuf_tensor(f"kmt{s}", [128, slot_nch[s] * 132], f16))
               for s in range(NS)]
        vvt = [es.enter_context(nc.sbuf_tensor(f"vvt{s}", [128, slot_nch[s] * 128], f16))
               for s in range(NS)]
        p0t = [es.enter_context(nc.sbuf_tensor(f"p0t{s}", [128, slot_nch[s] * 4], f16))
               for s in range(NS)]
        pt = [es.enter_context(nc.sbuf_tensor(f"pt{s}", [128, slot_nch[s] * 4], f16))
              for s in range(NS)]
        osb = es.enter_context(nc.sbuf_tensor("osbt", [128, 4 * NS], f32))
        ssb = es.enter_context(nc.sbuf_tensor("ssbt", [1, SUMW], f32))
        sct = [es.enter_context(nc.psum_tensor(f"sct{i}", [128, MAXW], f32))
               for i in range(4)]
        ovt = [es.enter_context(nc.psum_tensor(f"ovt{i}", [128, 4], f32))
               for i in range(2)]
        dst = [es.enter_context(nc.psum_tensor(f"dst{i}", [1, MAXW], f32))
               for i in range(2)]

        sq = es.enter_context(nc.semaphore(name="sq"))
        skm = [es.enter_context(nc.semaphore(name=f"skm{s}")) for s in range(NS)]
        svv = [es.enter_context(nc.semaphore(name=f"svv{s}")) for s in range(NS)]
        sqk = es.enter_context(nc.semaphore(name="sqk"))
        sexp = es.enter_context(nc.semaphore(name="sexp"))
        sp = es.enter_context(nc.semaphore(name="sp"))
        spv = es.enter_context(nc.semaphore(name="spv"))
        scp = es.enter_context(nc.semaphore(name="scp"))
        souts = es.enter_context(nc.semaphore(name="souts"))
        all_sems = [sq] + skm + svv + [sqk, sexp, sp, spv, scp, souts]

        with nc.Block() as block:

            @block.sync
            def _(sync):
                sync.dma_start(out=qt[:], in_=qd_d[:]).then_inc(sq, 16)
                for s in range(NS):
                    sync.dma_start(out=kmt[s][:], in_=km_d[s][:]).then_inc(skm[s], 16)
                sync.wait_ge(scp, NS)
                sync.dma_start(out=oud_d[:], in_=osb[:]).then_inc(souts, 16)

            @block.scalar
            def _(scalar):
                for s in range(NS):
                    scalar.dma_start(out=vvt[s][:], in_=vv_d[s][:]).then_inc(svv[s], 16)
                for s in range(NS):
                    nch = slot_nch[s]
                    scalar.wait_ge(sqk, s + 1)
                    scalar.activation(
                        p0t[s][:], sct[s % 4][:, :nch * 4],
                        mybir.ActivationFunctionType.Exp, scale=SM_SCALE,
                    ).then_inc(sexp)
                scalar.wait_ge(scp, NS)
                scalar.dma_start(out=sud_d[:], in_=ssb[:]).then_inc(souts, 16)

            @block.tensor
            def _(tensor):
                tensor.wait_ge(sq, 16)

                def qk(s):
                    nch = slot_nch[s]
                    tensor.wait_ge(skm[s], 16)
                    if s >= 4:
                        tensor.wait_ge(sexp, s - 3)   # sc bank WAR vs exp(s-4)
                    mm = None
                    for c in range(nch):
                        mm = tensor.matmul(
                            sct[s % 4][:, 4 * c:4 * c + 4],
                            kmt[s][:, 128 * c:128 * (c + 1)],
                            qt[:, 4 * s:4 * s + 4],
                            start=True, stop=True,
                        )
                    mm.then_inc(sqk)

                def pv(s):
                    nch = slot_nch[s]
                    tensor.wait_ge(sp, s + 1)
                    tensor.wait_ge(svv[s], 16)
                    if s >= 2:
                        tensor.wait_ge(scp, s - 1)    # ov/ds bank WAR vs copies(s-2)
                    for c in range(nch):
                        tensor.matmul(
                            ovt[s % 2][:],
                            vvt[s][:, 128 * c:128 * (c + 1)],
                            pt[s][:, 4 * c:4 * c + 4],
                            start=(c == 0), stop=(c == nch - 1),
                        )
                    tensor.matmul(
                        dst[s % 2][:1, :nch * 4], qt[:, 4 * NS:4 * NS + 1], pt[s][:],
                        start=True, stop=True,
                    ).then_inc(spv)

                qk(0)
                for s in range(1, NS):
                    qk(s)
                    pv(s - 1)
                pv(NS - 1)

            @block.vector
            def _(vector):
                def copies(j):
                    nch = slot_nch[j]
                    vector.wait_ge(spv, j + 1)
                    vector.tensor_copy(osb[:, 4 * j:4 * j + 4], ovt[j % 2][:])
                    vector.tensor_copy(
                        ssb[:, sum_off[j]:sum_off[j] + nch * 4],
                        dst[j % 2][:1, :nch * 4],
                    ).then_inc(scp)

                for s in range(NS):
                    nch = slot_nch[s]
                    vector.wait_ge(sexp, s + 1)
                    vector.tensor_mul(
                        pt[s][:], p0t[s][:],
                        kmt[s][:, nch * 128:nch * 132]).then_inc(sp)
                    if s >= 1:
                        copies(s - 1)
                copies(NS - 1)

            @block.gpsimd
            def _(gpsimd):
                gpsimd.wait_ge(souts, 32)
                for rng in bass.compact_to_ranges([h.num for h in all_sems]):
                    gpsimd.dma_reset(rng)
                    gpsimd.sem_clear(rng)

        nc.compile()
    return nc


def _prep(q, k_cache, v_cache, block_tables, context_lens, layout_crow, layout_col):
    """Resolve CSR rows, dedup kv blocks per (b, kv-head), build panels."""
    q_pid = context_lens.astype(np.int64) - 1            # [B]
    pbid = q_pid // BLK
    h_idx = np.arange(H)
    start = layout_crow[h_idx[None, :], pbid[:, None]]   # [B,H]
    end = layout_crow[h_idx[None, :], pbid[:, None] + 1]

    panels = []  # (nch, b, kv, U, cols_per_head)
    for b in range(B):
        for kv in range(KVH):
            cols_h = []
            for dh in range(GRP):
                h = kv * GRP + dh
                cols_h.append(layout_col[h, start[b, h]:end[b, h]])
            U = np.unique(np.concatenate(cols_h))
            nch = max(1, -(-(len(U) * BLK) // 128))
            panels.append((nch, b, kv, U, cols_h))

    order = sorted(range(len(panels)), key=lambda i: -panels[i][0])
    slot_nch = [0] * NSLOTS
    assign = [[None] * NSLOTS for _ in range(NC_CORES)]
    for rank, pi in enumerate(order):
        core, s = rank % NC_CORES, rank // NC_CORES
        assign[core][s] = pi
        if core == 0:
            slot_nch[s] = panels[pi][0]
    slot_nch = tuple(slot_nch)

    in_maps = []
    meta = []    # per core: list of (b, kv) per slot
    tok16 = np.arange(BLK)
    for core in range(NC_CORES):
        im = {}
        mt_core = []
        qd = np.zeros((128, 4 * NSLOTS + 1), np.float16)
        qd[:, 4 * NSLOTS] = 1.0
        for s in range(NSLOTS):
            nch, b, kv, U, cols_h = panels[assign[core][s]]
            NT = slot_nch[s] * 128
            NU = len(U)
            phys = block_tables[b, U]

            kmt = np.zeros((128, slot_nch[s] * 132), np.float16)
            kb = k_cache[phys, kv]                       # [NU, 32, 16, 4]
            kmt[:, :NU * BLK] = kb.transpose(1, 3, 0, 2).reshape(128, NU * BLK)

            vb = v_cache[phys, kv]                       # [NU, 128, 16]
            v_t = np.zeros((NT, 128), np.float16)
            v_t[:NU * BLK] = vb.transpose(0, 2, 1).reshape(NU * BLK, 128)
            vvt = np.ascontiguousarray(
                v_t.reshape(slot_nch[s], 128, 128).transpose(1, 0, 2)
                .reshape(128, NT))

            mm = np.zeros((4, NT), np.float16)
            upos = U * BLK
            causal = (upos[:, None] + tok16[None, :]) <= q_pid[b]   # [NU,16]
            for dh in range(GRP):
                allowed = np.isin(U, cols_h[dh])[:, None] & causal
                mm[dh, :NU * BLK] = allowed.reshape(-1).astype(np.float16)
            kmt[:, NT:] = (
                mm.reshape(4, slot_nch[s], 128).transpose(2, 1, 0)
                .reshape(128, slot_nch[s] * 4))

            im[f"km{s}"] = kmt
            im[f"vv{s}"] = vvt
            qd[:, 4 * s:4 * s + 4] = q[b, kv * GRP:(kv + 1) * GRP].T
            mt_core.append((b, kv))
        im["qd"] = qd
        in_maps.append(im)
        meta.append(mt_core)
    return slot_nch, in_maps, meta


def kernel(q, k_cache, v_cache, block_tables, context_lens, layout_crow, layout_col):
    import os
    from concourse.bass_utils import run_bass_kernel_spmd

    q = np.asarray(q, np.float32)
    k_cache = np.asarray(k_cache, np.float32)
    v_cache = np.asarray(v_cache, np.float32)
    block_tables = np.asarray(block_tables, np.int32)
    context_lens = np.asarray(context_lens, np.int32)
    layout_crow = np.asarray(layout_crow, np.int32)
    layout_col = np.asarray(layout_col, np.int32)

    slot_nch, in_maps, meta = _prep(
        q, k_cache, v_cache, block_tables, context_lens, layout_crow, layout_col)

    raw = bool(os.environ.get("KERNEL_RAW"))
    key = (slot_nch, raw)
    nc = _PROG_CACHE.get(key)
    if nc is None:
        build = _build_device_program_raw if raw else _build_device_program
        nc = build(slot_nch)
        _PROG_CACHE[key] = nc

    res = run_bass_kernel_spmd(
        nc, in_maps, core_ids=list(range(NC_CORES)),
        trace=bool(os.environ.get("KERNEL_TRACE")),
    )
    global _LAST_RESULT
    _LAST_RESULT = res

    out = np.empty((B, H, D), np.float32)
    for core in range(NC_CORES):
        oud = res.results[core]["oud"]                   # [128, 4*NSLOTS]
        sud = res.results[core]["sud"][0]                # [SUMW]
        off = 0
        for s in range(NSLOTS):
            nch = slot_nch[s]
            b, kv = meta[core][s]
            den = sud[off:off + nch * 4].reshape(nch, 4).sum(0)   # [4]
            out[b, kv * GRP:(kv + 1) * GRP] = (oud[:, 4 * s:4 * s + 4] / den).T
            off += nch * 4
    return out


_LAST_RESULT = None
